# revision 1
# baseline (speedup 1.0000x reference)
"""Causal single-head attention (B=4, S=2048, D=768) on 8 trn2 NeuronCores.

Sharding: batch (4) x query-split (2). Core c = 2*b + r handles batch b and
the 8 interleaved query blocks {2i+r : i=0..7} (128 rows each). Both cores of
a batch pair compute the full K/V projections (duplicate compute, no
collectives).

Pipeline per core (matmuls as out = lhsT.T @ rhs, fp32 viewed as float32r):
  Phase Q : QT[dout, q]  = Wq.T @ XTq        (co-major, 512-col psum groups)
  Phase K : KT[dout, s]  = Wk.T @ XT         (g-major so ST can chase)
  Phase V : V[s, dout+1] = [XT.T @ Wv | 1]   (ones col -> softmax row sums)
  Phase A : per key block j: ST_j[k, q-suffix] = KT_j.T-style matmul
            (scores computed directly TRANSPOSED -> no PE transposes),
            +mask (diagonal tril / pad kill, role-specific DATA only),
            exp -> PT_j in SBUF; per q slot i: O_i = sum_j PT_j^T @ V'_j
            accumulated in PSUM; last column = row sum; scale by reciprocal.

Role asymmetry (which key block is diagonal / padded) is carried entirely by
the mask INPUT so the same SPMD program runs on all 8 cores. Slot i is padded
to E_i = 2i+2 key blocks; the pad block's P rows are zeroed by a -1e30 mask.
"""

import os
import sys

for _p in ("/opt/trn_rl_repo", "/root/.axon_site/_ro/trn_rl_repo"):
    if os.path.isdir(_p) and _p not in sys.path:
        sys.path.append(_p)

import numpy as np

import concourse.bacc as bacc
import concourse.mybir as mybir
import concourse.tile as tile
from concourse._compat import get_trn_type

B, S, D = 4, 2048, 768
P = 128
DC = D // P          # 6 contraction / dout chunks
SB = S // P          # 16 seq blocks
NQ = 8               # q-slots per core
QW = NQ * P          # 1024 q rows per core
QWP = QW + P         # qt free width incl 128 zero-pad cols
SCALE = 1.0 / float(np.sqrt(D))
MASK_VAL = -1e30

F32 = mybir.dt.float32
F32R = mybir.dt.float32r
BF16 = mybir.dt.bfloat16
USE_BF16 = True
DT_IN = BF16 if USE_BF16 else F32R

# Padded key-suffix width (in 128-blocks) of ST_j across both roles, and the
# first q-slot of that suffix.
W_BLK = [8 - (j // 2) for j in range(SB)]          # [8,8,7,7,...,1,1]
S0 = [j // 2 for j in range(SB)]


def _st_pieces(j):
    """PSUM column pieces for ST_j: each >=256 wide, never straddling a
    2KB bank boundary. Pad pieces read qt's zeroed cols [1024:1152)."""
    w = W_BLK[j] * P
    if w <= 512:
        return [(0, max(w, 256))]
    return [(0, 512), (512, max(w - 512, 256))]


def build_nc(reps=1):
    nc = bacc.Bacc(
        get_trn_type() or "TRN2",
        target_bir_lowering=False,
        debug=False,
        num_devices=8,
        dynamic_dma_scratch_size=2048,
    )
    xt_d = nc.dram_tensor("xt", [D, S], DT_IN, kind="ExternalInput").ap()
    xtq_d = nc.dram_tensor("xtq", [D, QW], DT_IN, kind="ExternalInput").ap()
    # wq/wk are packed co-major on host: row block co holds, side by side,
    # the [128,128] tiles W[ci*128:(ci+1)*128, co*128:(co+1)*128] for ci=0..5.
    wq_d = nc.dram_tensor("wq", [D, D], DT_IN, kind="ExternalInput").ap()
    wk_d = nc.dram_tensor("wk", [D, D], DT_IN, kind="ExternalInput").ap()
    wv_d = nc.dram_tensor("wv", [D, D], DT_IN, kind="ExternalInput").ap()
    mask_d = nc.dram_tensor("mask", [SB, P, P], F32, kind="ExternalInput").ap()
    o_d = nc.dram_tensor("o", [QW, D], F32, kind="ExternalOutput").ap()

    for _rep in range(reps):
        _emit_body(nc, xt_d, xtq_d, wq_d, wk_d, wv_d, mask_d, o_d)
    return nc


def _emit_body(nc, xt_d, xtq_d, wq_d, wk_d, wv_d, mask_d, o_d):
    with tile.TileContext(nc) as tc:
        persist = tc.alloc_tile_pool(name="persist", bufs=1)
        qt = [persist.tile([P, QWP], DT_IN, tag=f"qt{c}", name=f"qt{c}")
              for c in range(DC)]
        masks = [persist.tile([P, P], F32, tag=f"m{j}", name=f"m{j}")
                 for j in range(SB)]
        onez = persist.tile([P, 2], F32, tag="onez", name="onez")
        nc.gpsimd.memset(onez[:, 0:1], 1.0)
        nc.gpsimd.memset(onez[:, 1:2], 0.0)

        # zero the q pad cols once; ST pad pieces read them.
        for c in range(DC):
            nc.gpsimd.memset(qt[c][:, QW:QWP].bitcast(F32), 0.0)

        xt_pool = tc.alloc_tile_pool(name="xt_pool", bufs=1)
        xt = [xt_pool.tile([P, S], DT_IN, tag=f"xt{c}", name=f"xt{c}")
              for c in range(DC)]

        # ---------------- input DMAs, in consumption order, one queue ------
        # pool alloc order is release order reversed (stack discipline):
        # releases are xtq,wq (end Q); wk (end K); wv,xt (end V).
        wv_pool = tc.alloc_tile_pool(name="wv_pool", bufs=1)
        wv = [wv_pool.tile([P, D], DT_IN, tag=f"wv{c}", name=f"wv{c}")
              for c in range(DC)]
        wk_pool = tc.alloc_tile_pool(name="wk_pool", bufs=1)
        wkt = [wk_pool.tile([P, D], DT_IN, tag=f"wkt{c}", name=f"wkt{c}")
               for c in range(DC)]
        wq_pool = tc.alloc_tile_pool(name="wq_pool", bufs=1)
        wqt = [wq_pool.tile([P, D], DT_IN, tag=f"wqt{c}", name=f"wqt{c}")
               for c in range(DC)]
        xtq_pool = tc.alloc_tile_pool(name="xtq_pool", bufs=1)
        xtq = [xtq_pool.tile([P, QW], DT_IN, tag=f"xtq{c}", name=f"xtq{c}")
               for c in range(DC)]

        for c in range(DC):
            nc.sync.dma_start(xtq[c][:], xtq_d[c * P:(c + 1) * P, :])
        for c in range(DC):
            nc.sync.dma_start(wqt[c][:], wq_d[c * P:(c + 1) * P, :])
        # xt in 512-col groups interleaved with wk so K proj can chase.
        for sg in range(2):
            for c in range(DC):
                nc.scalar.dma_start(xt[c][:, sg * 512:(sg + 1) * 512],
                                    xt_d[c * P:(c + 1) * P, sg * 512:(sg + 1) * 512])
        for c in range(DC):
            nc.sync.dma_start(wkt[c][:], wk_d[c * P:(c + 1) * P, :])
        for sg in range(2, 4):
            for c in range(DC):
                nc.scalar.dma_start(xt[c][:, sg * 512:(sg + 1) * 512],
                                    xt_d[c * P:(c + 1) * P, sg * 512:(sg + 1) * 512])
        for c in range(DC):
            nc.sync.dma_start(wv[c][:], wv_d[c * P:(c + 1) * P, :])
        for j in range(SB):
            nc.sync.dma_start(masks[j][:], mask_d[j])

        # ---------------- Phase Q: qt[co][:, g] = sum_ci Wq^T Xq ----------
        with tc.tile_pool(name="psum_q", bufs=4, space="PSUM") as ppq:
            for co in range(DC):
                for g in range(QW // 512):
                    ps = ppq.tile([P, 512], F32, tag="pq", name="pq")
                    for ci in range(DC):
                        nc.tensor.matmul(
                            ps[:],
                            wqt[co][:, ci * P:(ci + 1) * P],
                            xtq[ci][:, g * 512:(g + 1) * 512],
                            start=(ci == 0), stop=(ci == DC - 1),
                        )
                    nc.scalar.copy(qt[co][:, g * 512:(g + 1) * 512], ps[:])
        xtq_pool.release()
        wq_pool.release()

        # ---------------- Phase K: g-major ---------------------------------
        kt_pool = tc.alloc_tile_pool(name="kt_pool", bufs=1, side="right")
        kt = [kt_pool.tile([P, S], DT_IN, tag=f"kt{c}", name=f"kt{c}")
              for c in range(DC)]
        with tc.tile_pool(name="psum_k", bufs=4, space="PSUM") as ppk:
            for g in range(S // 512):
                for co in range(DC):
                    ps = ppk.tile([P, 512], F32, tag="pk", name="pk")
                    for ci in range(DC):
                        nc.tensor.matmul(
                            ps[:],
                            wkt[co][:, ci * P:(ci + 1) * P],
                            xt[ci][:, g * 512:(g + 1) * 512],
                            start=(ci == 0), stop=(ci == DC - 1),
                        )
                    if co % 2 == 0:
                        nc.scalar.copy(kt[co][:, g * 512:(g + 1) * 512], ps[:])
                    else:
                        nc.vector.tensor_copy(kt[co][:, g * 512:(g + 1) * 512], ps[:])
        wk_pool.release()

        # ---------------- Phase V ------------------------------------------
        v_pool = tc.alloc_tile_pool(name="v_pool", bufs=1, side="right")
        v = [v_pool.tile([P, D + 2], DT_IN, tag=f"v{j}", name=f"v{j}")
             for j in range(SB)]
        with tc.tile_pool(name="psum_v", bufs=3, space="PSUM") as ppv:
            for j in range(SB):
                ps = ppv.tile([P, D], F32, tag="pv", name="pv")
                for (n0, nw) in ((0, 512), (512, 256)):
                    for ci in range(DC):
                        nc.tensor.matmul(
                            ps[:, n0:n0 + nw],
                            xt[ci][:, j * P:(j + 1) * P],
                            wv[ci][:, n0:n0 + nw],
                            start=(ci == 0), stop=(ci == DC - 1),
                        )
                nc.vector.tensor_copy(v[j][:, 0:D], ps[:])
                nc.vector.tensor_copy(v[j][:, D:D + 2], onez[:])
        wv_pool.release()
        xt_pool.release()

        # ---------------- Phase A: transposed scores + PV ------------------
        pt_pool = tc.alloc_tile_pool(name="pt_pool", bufs=1, side="right")
        pt = []
        for j in range(SB):
            pcs = _st_pieces(j)
            ptw = pcs[-1][0] + pcs[-1][1]
            pt.append(pt_pool.tile([P, ptw], DT_IN, tag=f"pt{j}", name=f"pt{j}"))

        with (
            tc.tile_pool(name="psum_st", bufs=2, space="PSUM") as pst,
            tc.tile_pool(name="psum_o", bufs=2, space="PSUM") as pso,
            tc.tile_pool(name="o_sb", bufs=2) as o_pool,
            tc.tile_pool(name="small", bufs=4) as small,
        ):
            def emit_st(j):
                qbase = S0[j] * P
                st = pst.tile([P, 1024], F32, tag="st", name="st")
                pcs = _st_pieces(j)
                for (p0, pw) in pcs:
                    for ci in range(DC):
                        nc.tensor.matmul(
                            st[:, p0:p0 + pw],
                            kt[ci][:, j * P:(j + 1) * P],
                            qt[ci][:, qbase + p0:qbase + p0 + pw],
                            start=(ci == 0), stop=(ci == DC - 1),
                        )
                nc.vector.tensor_add(st[:, 0:P], st[:, 0:P], masks[j][:])
                for (p0, pw) in pcs:
                    nc.scalar.activation(
                        pt[j][:, p0:p0 + pw], st[:, p0:p0 + pw],
                        mybir.ActivationFunctionType.Exp, scale=SCALE,
                    )

            def emit_pv(i):
                e = 2 * i + 2
                po = pso.tile([P, 1024], F32, tag="po", name="po")
                for j in range(e):
                    lhs = pt[j][:, (i - S0[j]) * P:(i - S0[j] + 1) * P]
                    for (n0, nw) in ((0, 512), (512, D + 2 - 512)):
                        nc.tensor.matmul(
                            po[:, n0:n0 + nw], lhs, v[j][:, n0:n0 + nw],
                            start=(j == 0), stop=(j == e - 1),
                        )
                rec = small.tile([P, 1], F32, tag="rec", name="rec")
                nc.vector.reciprocal(rec[:], po[:, D:D + 1])
                osb = o_pool.tile([P, D], F32, tag="osb", name="osb")
                nc.scalar.activation(
                    osb[:], po[:, 0:D], mybir.ActivationFunctionType.Copy,
                    scale=rec[:, 0:1],
                )
                nc.sync.dma_start(o_d[i * P:(i + 1) * P, :], osb[:])

            # interleave: PV_i emitted after ST_{2i+3} so exp has slack.
            emitted_pv = 0
            for j in range(SB):
                emit_st(j)
                while emitted_pv < NQ and j >= min(2 * emitted_pv + 3, SB - 1):
                    emit_pv(emitted_pv)
                    emitted_pv += 1
            while emitted_pv < NQ:
                emit_pv(emitted_pv)
                emitted_pv += 1

        pt_pool.release()
        v_pool.release()
        kt_pool.release()
        persist.release()


# ---------------------------------------------------------------------------
# host side

def _build_masks():
    """masks[r][j] : [128,128] additive mask for role r, key block j, applied
    to the first 128 q-cols of ST_j (slot s0 = j//2, global q block 2*s0+r).

    g == j  -> causal tril (allow k <= q)
    g <  j  -> pad slot: kill the whole block
    g >  j  -> fully allowed
    """
    tril = np.tril(np.full((P, P), MASK_VAL, np.float32), -1)  # kill k > q
    out = []
    for r in (0, 1):
        m = np.zeros((SB, P, P), np.float32)
        for j in range(SB):
            g = 2 * S0[j] + r
            if g == j:
                m[j] = tril
            elif g < j:
                m[j, :, :] = MASK_VAL
        out.append(m)
    return out


def _pack_co_major(w):
    """[768,768] -> row block co holds [W[ci*128:(ci+1)*128, co*128:(co+1)*128]
    for ci=0..5] side by side."""
    blocks = []
    for co in range(DC):
        blocks.append(np.concatenate(
            [w[ci * P:(ci + 1) * P, co * P:(co + 1) * P] for ci in range(DC)],
            axis=1,
        ))
    return np.ascontiguousarray(np.concatenate(blocks, axis=0))


_STATE = {}


def _get_nc():
    if "nc" not in _STATE:
        nc = build_nc()
        nc.finalize()
        _STATE["nc"] = nc
    return _STATE["nc"]


def _cast_in(a):
    if not USE_BF16:
        return np.ascontiguousarray(a)
    import ml_dtypes
    return np.ascontiguousarray(a.astype(ml_dtypes.bfloat16))


def build_in_maps(x, Wq, Wk, Wv):
    masks = _build_masks()
    wqp = _cast_in(_pack_co_major(Wq))
    wkp = _cast_in(_pack_co_major(Wk))
    Wv = _cast_in(Wv)
    in_maps = []
    for b in range(B):
        xt = np.ascontiguousarray(x[b].T)            # [768, 2048]
        for r in (0, 1):
            cols = np.concatenate(
                [xt[:, (2 * i + r) * P:(2 * i + r + 1) * P] for i in range(NQ)],
                axis=1,
            )
            in_maps.append({
                "xt": _cast_in(xt),
                "xtq": _cast_in(cols),
                "wq": wqp, "wk": wkp, "wv": Wv,
                "mask": masks[r],
            })
    return in_maps


def kernel(x, Wq, Wk, Wv):
    x = np.ascontiguousarray(np.asarray(x, np.float32))
    Wq = np.ascontiguousarray(np.asarray(Wq, np.float32))
    Wk = np.ascontiguousarray(np.asarray(Wk, np.float32))
    Wv = np.ascontiguousarray(np.asarray(Wv, np.float32))

    from concourse.bass_utils import run_bass_kernel_spmd

    nc = _get_nc()
    in_maps = build_in_maps(x, Wq, Wk, Wv)

    res = run_bass_kernel_spmd(nc, in_maps, core_ids=list(range(8)), trace=False)

    out = np.empty((B, S, D), np.float32)
    for b in range(B):
        for r in (0, 1):
            o = res.results[2 * b + r]["o"]
            for i in range(NQ):
                g = 2 * i + r
                out[b, g * P:(g + 1) * P, :] = o[i * P:(i + 1) * P, :]
    return out


# ---------------------------------------------------------------------------
# benchmarking support (not used by the grading path)

def _make_executor(nc, n_cores=8):
    """Build a cached jitted SPMD callable (no donation, reusable buffers)."""
    import jax
    from jax.sharding import Mesh, PartitionSpec
    try:
        from jax.experimental.shard_map import shard_map
    except ImportError:
        from jax.shard_map import shard_map
    from concourse import bass2jax
    from concourse import mybir as mb

    bass2jax.install_neuronx_cc_hook()
    partition_name = nc.partition_id_tensor.name if nc.partition_id_tensor else None
    in_names, out_names, out_avals, zero_outs = [], [], [], []
    for alloc in nc.m.functions[0].allocations:
        if not isinstance(alloc, mb.MemoryLocationSet):
            continue
        name = alloc.memorylocations[0].name
        if alloc.kind == "ExternalInput":
            if name != partition_name:
                in_names.append(name)
        elif alloc.kind == "ExternalOutput":
            shape = tuple(alloc.tensor_shape)
            dtype = mb.dt.np(alloc.dtype)
            out_names.append(name)
            out_avals.append(jax.core.ShapedArray(shape, dtype))
            zero_outs.append(np.zeros(shape, dtype))
    n_params = len(in_names)
    all_names = list(in_names) + list(out_names)
    if partition_name is not None:
        all_names.append(partition_name)

    def _body(*args):
        operands = list(args)
        if partition_name is not None:
            operands.append(bass2jax.partition_id_tensor())
        outs = bass2jax._bass_exec_p.bind(
            *operands,
            out_avals=tuple(out_avals),
            in_names=tuple(all_names),
            out_names=tuple(out_names),
            lowering_input_output_aliases=(),
            sim_require_finite=True,
            sim_require_nnan=True,
            nc=nc,
        )
        return tuple(outs)

    devices = jax.devices()[:n_cores]
    mesh = Mesh(np.asarray(devices), ("core",))
    in_specs = (PartitionSpec("core"),) * (n_params + len(out_names))
    out_specs = (PartitionSpec("core"),) * len(out_names)
    sharded = jax.jit(
        shard_map(_body, mesh=mesh, in_specs=in_specs, out_specs=out_specs,
                  check_rep=False),
        keep_unused=True,
    )
    return sharded, in_names, out_names, out_avals, zero_outs


def benchmark(in_maps, iters=20, n_cores=8):
    """Run the compiled kernel `iters` times back-to-back on device-resident
    inputs; returns (per_iter_seconds, results_core0_dict)."""
    import time as _time
    import jax

    nc = _get_nc()
    key = "exec"
    if key not in _STATE:
        _STATE[key] = _make_executor(nc, n_cores)
    sharded, in_names, out_names, out_avals, zero_outs = _STATE[key]

    concat_in = [
        np.concatenate([np.asarray(in_maps[c][n]) for c in range(n_cores)], axis=0)
        for n in in_names
    ]
    concat_zeros = [
        np.zeros((n_cores * z.shape[0], *z.shape[1:]), z.dtype) for z in zero_outs
    ]
    args = [jax.device_put(a) for a in concat_in + concat_zeros]
    jax.block_until_ready(args)

    outs = sharded(*args)          # warm-up / compile
    jax.block_until_ready(outs)
    t0 = _time.time()
    for _ in range(iters):
        outs = sharded(*args)
    jax.block_until_ready(outs)
    per_iter = (_time.time() - t0) / iters
    res0 = {
        n: np.asarray(outs[i]).reshape(n_cores, *out_avals[i].shape)[0]
        for i, n in enumerate(out_names)
    }
    return per_iter, res0


def measure_exec_ns(iters=12, reps_pair=(64, 96)):
    """Estimate true per-core HW execution time of one kernel body.

    Per-call wall time through the axon tunnel is dominated by a transfer
    floor proportional to I/O bytes (~16 ms) that completely hides execution.
    So we build NEFFs with the body repeated r1/r2 times (same I/O footprint)
    and use the slope: (wall(r2) - wall(r1)) / (r2 - r1).
    """
    import time as _time
    import jax

    rng = np.random.default_rng(0)
    x = rng.standard_normal((B, S, D)).astype(np.float32)
    sc = 1.0 / np.sqrt(D)
    Wq = rng.uniform(-sc, sc, (D, D)).astype(np.float32)
    Wk = rng.uniform(-sc, sc, (D, D)).astype(np.float32)
    Wv = rng.uniform(-sc, sc, (D, D)).astype(np.float32)
    in_maps = build_in_maps(x, Wq, Wk, Wv)

    pers = {}
    for reps in reps_pair:
        nc = build_nc(reps=reps)
        nc.finalize()
        sharded, in_names, out_names, out_avals, zero_outs = _make_executor(nc, 8)
        concat_in = [
            np.concatenate([np.asarray(in_maps[c][n]) for c in range(8)], axis=0)
            for n in in_names
        ]
        concat_zeros = [
            np.zeros((8 * z.shape[0], *z.shape[1:]), z.dtype) for z in zero_outs
        ]
        args = [jax.device_put(a) for a in concat_in + concat_zeros]
        jax.block_until_ready(args)
        outs = sharded(*args)
        jax.block_until_ready(outs)
        best = None
        for _ in range(3):
            t0 = _time.time()
            for _ in range(iters):
                outs = sharded(*args)
            jax.block_until_ready(outs)
            per = (_time.time() - t0) / iters
            best = per if best is None else min(best, per)
        pers[reps] = best
    r1, r2 = reps_pair
    return int((pers[r2] - pers[r1]) / (r2 - r1) * 1e9)

